# revision 6
# baseline (speedup 1.0000x reference)
"""CategorySpecificLinear Trainium2 kernel.

out[t] = x[t] @ weight[category_id[t]] + bias[category_id[t]]

Strategy: expert-parallel over the 8 categories (C == n_cores == 8).
Host routes tokens by category, transposes each category's token block
to [D, T_pad] and casts x/w to bf16 (fp32 accumulate in PSUM keeps the
rel err ~1e-3, far under the 2e-2 gate). Core c computes
    out = xT.T @ w + bias    (out in bf16, host casts back to fp32)

v2 vs the fp32r baseline (44.2us):
  - bf16 halves HBM traffic (3.4 MB/core vs 9.2) and matmul cost.
  - n=0 pass runs k-outer/m-inner so the PE consumes each k-slice as
    its DMA lands (x_k 0.27 MB + w_k 0.25 MB per slice); n=1 pass runs
    m-outer/k-inner so psum groups complete staggered and the bias-add
    + output DMA drain overlaps compute instead of piling up at the end.
  - out is written as one contiguous [m, 1024] bf16 DMA per m-tile.
  - fewer instructions => fewer tile semaphores => shorter framework
    wind-down epilogue (was ~9us of per-semaphore waits).
"""

import contextlib
import ctypes
import os
import sys
import types

import numpy as np
import ml_dtypes

sys.path.insert(0, "/opt/trn_rl_repo")

BF16 = np.dtype(ml_dtypes.bfloat16)


def _ensure_ntff_hook():
    """Provide antenv.axon_hooks if the image lacks it.

    concourse.bass_utils imports antenv.axon_hooks.get_axon_ntff_profile_hook
    when trace=True under axon; some agent images don't ship that module, in
    which case the boot's NTFF hook registration silently degrades and the
    import in bass_utils crashes. Recreate the slim ctypes hook here
    (mirrors trn_agent_boot.trn_boot._ntff_profile_via_ctypes).
    """
    try:
        import antenv.axon_hooks  # noqa: F401

        return
    except ImportError:
        pass

    so_path = "/opt/axon/libaxon_pjrt.so"
    hook = None
    if os.path.exists(so_path):
        lib = ctypes.CDLL(so_path)
        if hasattr(lib, "axon_start_nrt_profile"):
            lib.axon_start_nrt_profile.argtypes = [
                ctypes.POINTER(ctypes.c_int64),
                ctypes.c_size_t,
            ]
            lib.axon_start_nrt_profile.restype = ctypes.c_int64
            lib.axon_stop_nrt_profile.argtypes = [ctypes.c_char_p]
            lib.axon_stop_nrt_profile.restype = ctypes.c_int64

            @contextlib.contextmanager
            def hook(output_dir, device_ids):
                import jax

                jax.devices()
                if device_ids:
                    ids = (ctypes.c_int64 * len(device_ids))(*device_ids)
                    rc = lib.axon_start_nrt_profile(ids, len(device_ids))
                else:
                    rc = lib.axon_start_nrt_profile(None, 0)
                if rc != 0:
                    raise RuntimeError(f"axon_start_nrt_profile rc={rc}")
                try:
                    yield
                finally:
                    n = lib.axon_stop_nrt_profile(str(output_dir).encode())
                    if n <= 0:
                        print(
                            f"ntff profile: rc={n} writing {output_dir}",
                            file=sys.stderr,
                        )

    mod = types.ModuleType("antenv.axon_hooks")
    _state = {"hook": hook}
    mod.set_axon_ntff_profile_hook = lambda h: _state.__setitem__("hook", h)
    mod.get_axon_ntff_profile_hook = lambda: _state["hook"]
    sys.modules["antenv.axon_hooks"] = mod
    try:
        import antenv

        antenv.axon_hooks = mod
    except ImportError:
        pass


_ensure_ntff_hook()

import concourse.bass as bass
import concourse.bacc as bacc_mod
import concourse.mybir as mybir
import concourse.tile as tile
from concourse.bass import ts
from concourse.bass_utils import run_bass_kernel_spmd

N_CORES = 8
P = 128
N_TILE = 512  # one fp32 PSUM bank

_nc_cache = {}
LAST_RESULTS = None  # BassKernelResults of the most recent run (for test.py)


def _build_nc(T_pad: int, D: int, O: int):
    KO = D // P
    NO = O // N_TILE
    bf16 = mybir.dt.bfloat16
    f32 = mybir.dt.float32

    # m-tiles: full 128-row tiles plus one remainder tile (multiple of 32)
    m_sizes = [P] * (T_pad // P)
    if T_pad % P:
        m_sizes.append(T_pad % P)
    MO = len(m_sizes)
    m_starts = [sum(m_sizes[:i]) for i in range(MO)]

    nc = bacc_mod.Bacc()
    xT = nc.dram_tensor("xT", [D, T_pad], bf16, kind="ExternalInput")
    w = nc.dram_tensor("w", [D, O], bf16, kind="ExternalInput")
    bias = nc.dram_tensor("bias", [1, O], f32, kind="ExternalInput")
    out = nc.dram_tensor("out", [T_pad, O], bf16, kind="ExternalOutput")

    xT_t = xT[:, :].rearrange("(ko p) t -> p ko t", p=P)
    w_t = w[:, :].rearrange("(ko p) o -> p ko o", p=P)

    with tile.TileContext(nc) as tc:
        with (
            tc.tile_pool(name="resident", bufs=1) as rpool,
            tc.tile_pool(name="psum", bufs=8, space="PSUM") as psum_pool,
            tc.tile_pool(name="obuf", bufs=MO) as opool,
        ):
            # HAM warm-up: dummy matmuls lift the PE clock gate to 8/8
            # before the real stream starts. Each bass-level warm matmul
            # lowers to 2 MATMUL instructions (measured), so 12 calls =
            # 24 x ~107 ns = ~2.6 us of PE activity. The operand tile is
            # deliberately NOT memset: the product is never read, and
            # skipping the memset removes the vector-engine gate so
            # warm-up starts ~1 us earlier.
            warm_sb = rpool.tile([P, 64], f32, tag="warm")
            nc.gpsimd.memset(warm_sb[:], 0.0)
            warm_ps = psum_pool.tile([64, 64], f32, tag="ps", name="warm_ps")
            for i in range(12):
                nc.tensor.matmul(
                    warm_ps[:],
                    lhsT=warm_sb[:, :64],
                    rhs=warm_sb[:, :64],
                    start=True,
                    stop=True,
                )
            # Input loads: k-pair granularity ([128, 2, T_pad/O] bf16 via
            # rearranged APs) halves the number of ~0.7 us DMA issues on
            # the two HWDGE queues, so the last input slice is issued by
            # ~9.5 us and lands at the ~358 GB/s HBM floor (~17 us).
            # bias broadcast rides the idle gpsimd (SWDGE) queue.
            bias_sb = rpool.tile([P, O], f32, tag="bias")
            x_sb = []
            w_sb = []
            for j in range(KO // 2):
                xt = rpool.tile([P, 2 * T_pad], bf16, tag=f"x{j}")
                wt = rpool.tile([P, 2 * O], bf16, tag=f"w{j}")
                xt3 = xt[:].rearrange("p (ko t) -> p ko t", ko=2)
                wt3 = wt[:].rearrange("p (ko o) -> p ko o", ko=2)
                if j % 2 == 0:
                    nc.sync.dma_start(xt3, xT_t[:, 2 * j : 2 * j + 2, :])
                    nc.scalar.dma_start(wt3, w_t[:, 2 * j : 2 * j + 2, :])
                else:
                    nc.scalar.dma_start(xt3, xT_t[:, 2 * j : 2 * j + 2, :])
                    nc.sync.dma_start(wt3, w_t[:, 2 * j : 2 * j + 2, :])
                x_sb.append(xt)
                w_sb.append(wt)
                if j == 0:
                    nc.gpsimd.dma_start(
                        bias_sb[:], bias[:, :].to_broadcast((P, O))
                    )

            def x_ap(k, m):
                return x_sb[k // 2][
                    :,
                    (k % 2) * T_pad + m_starts[m] : (k % 2) * T_pad
                    + m_starts[m]
                    + m_sizes[m],
                ]

            def w_ap(k, n):
                return w_sb[k // 2][
                    :, (k % 2) * O + n * N_TILE : (k % 2) * O + (n + 1) * N_TILE
                ]

            obufs = [
                opool.tile([P, O], bf16, tag="ot", name=f"ot{m}")
                for m in range(MO)
            ]

            # Pass n=0: k-outer / m-inner. The PE touches k-slice k for
            # ~1.1 us (MO matmuls) which matches the DMA delivery rate,
            # so compute ramps with the loads instead of stalling on the
            # full 3.1 MB. All MO psum groups accumulate in lockstep.
            ps0 = [
                psum_pool.tile([m_sizes[m], N_TILE], f32, tag="ps", name=f"ps0_{m}")
                for m in range(MO)
            ]
            for k in range(KO):
                for m in range(MO):
                    nc.tensor.matmul(
                        ps0[m][:],
                        lhsT=x_ap(k, m),
                        rhs=w_ap(k, 0),
                        start=(k == 0),
                        stop=(k == KO - 1),
                    )
            for m in range(MO):
                nc.vector.tensor_add(
                    obufs[m][: m_sizes[m], ts(0, N_TILE)],
                    ps0[m][:],
                    bias_sb[: m_sizes[m], ts(0, N_TILE)],
                )

            # Pass n=1: m-outer / k-inner (inputs are all resident by
            # now). Each m-tile's psum group completes ~1.7 us apart, so
            # the bias-add and the single contiguous [m, O] output DMA
            # overlap the remaining matmuls. The last tile is the small
            # remainder, keeping the post-matmul tail ~1 us.
            for m in range(MO):
                ps = psum_pool.tile(
                    [m_sizes[m], N_TILE], f32, tag="ps", name=f"ps1_{m}"
                )
                for k in range(KO):
                    nc.tensor.matmul(
                        ps[:],
                        lhsT=x_ap(k, m),
                        rhs=w_ap(k, 1),
                        start=(k == 0),
                        stop=(k == KO - 1),
                    )
                nc.vector.tensor_add(
                    obufs[m][: m_sizes[m], ts(1, N_TILE)],
                    ps[:],
                    bias_sb[: m_sizes[m], ts(1, N_TILE)],
                )
                eng = nc.sync if m % 2 == 0 else nc.scalar
                eng.dma_start(
                    out[m_starts[m] : m_starts[m] + m_sizes[m], :],
                    obufs[m][: m_sizes[m], :],
                )
    nc.finalize()
    return nc


def kernel(x, category_id, weight, bias):
    global LAST_RESULTS
    x = np.asarray(x)
    category_id = np.asarray(category_id)
    weight = np.asarray(weight, dtype=np.float32)
    bias = np.ascontiguousarray(np.asarray(bias), dtype=np.float32)

    orig_shape = x.shape
    D = orig_shape[-1]
    C, _, O = weight.shape
    assert C == N_CORES and D % P == 0 and O % N_TILE == 0

    T = int(np.prod(orig_shape[:-1]))
    x_flat = np.ascontiguousarray(x.reshape(T, D), dtype=np.float32)
    cid = category_id.reshape(T).astype(np.int64)

    idx_per_c = [np.flatnonzero(cid == c) for c in range(C)]
    counts = [len(ix) for ix in idx_per_c]
    T_pad = max(32, -(-max(counts) // 32) * 32)  # multiple of 32 (PE col-group)

    key = (T_pad, D, O)
    if key not in _nc_cache:
        _nc_cache[key] = _build_nc(T_pad, D, O)
    nc = _nc_cache[key]

    w_bf16 = weight.astype(BF16)
    in_maps = []
    for c in range(C):
        xcT = np.zeros((D, T_pad), dtype=BF16)
        xcT[:, : counts[c]] = x_flat[idx_per_c[c]].T.astype(BF16)
        in_maps.append(
            {
                "xT": xcT,
                "w": w_bf16[c],
                "bias": bias[c : c + 1],
            }
        )

    res = run_bass_kernel_spmd(nc, in_maps, list(range(N_CORES)))
    LAST_RESULTS = res

    out_flat = np.empty((T, O), dtype=np.float32)
    for c in range(C):
        out_flat[idx_per_c[c]] = res.results[c]["out"][: counts[c]].astype(
            np.float32
        )
    return out_flat.reshape(*orig_shape[:-1], O)


# revision 10
# speedup vs baseline: 1.0875x; 1.0875x over previous
"""CategorySpecificLinear Trainium2 kernel.

out[t] = x[t] @ weight[category_id[t]] + bias[category_id[t]]

Strategy: expert-parallel over the 8 categories (C == n_cores == 8).
Host routes tokens by category, transposes each category's token block
to [D, T_pad] and casts x/w to bf16 (fp32 accumulate in PSUM keeps the
rel err ~1e-3, far under the 2e-2 gate). Core c computes
    out = xT.T @ w + bias    (out in bf16, host casts back to fp32)

v2 vs the fp32r baseline (44.2us):
  - bf16 halves HBM traffic (3.4 MB/core vs 9.2) and matmul cost.
  - n=0 pass runs k-outer/m-inner so the PE consumes each k-slice as
    its DMA lands (x_k 0.27 MB + w_k 0.25 MB per slice); n=1 pass runs
    m-outer/k-inner so psum groups complete staggered and the bias-add
    + output DMA drain overlaps compute instead of piling up at the end.
  - out is written as one contiguous [m, 1024] bf16 DMA per m-tile.
  - fewer instructions => fewer tile semaphores => shorter framework
    wind-down epilogue (was ~9us of per-semaphore waits).
"""

import contextlib
import ctypes
import os
import sys
import types

import numpy as np
import ml_dtypes

sys.path.insert(0, "/opt/trn_rl_repo")

BF16 = np.dtype(ml_dtypes.bfloat16)


def _ensure_ntff_hook():
    """Provide antenv.axon_hooks if the image lacks it.

    concourse.bass_utils imports antenv.axon_hooks.get_axon_ntff_profile_hook
    when trace=True under axon; some agent images don't ship that module, in
    which case the boot's NTFF hook registration silently degrades and the
    import in bass_utils crashes. Recreate the slim ctypes hook here
    (mirrors trn_agent_boot.trn_boot._ntff_profile_via_ctypes).
    """
    try:
        import antenv.axon_hooks  # noqa: F401

        return
    except ImportError:
        pass

    so_path = "/opt/axon/libaxon_pjrt.so"
    hook = None
    if os.path.exists(so_path):
        lib = ctypes.CDLL(so_path)
        if hasattr(lib, "axon_start_nrt_profile"):
            lib.axon_start_nrt_profile.argtypes = [
                ctypes.POINTER(ctypes.c_int64),
                ctypes.c_size_t,
            ]
            lib.axon_start_nrt_profile.restype = ctypes.c_int64
            lib.axon_stop_nrt_profile.argtypes = [ctypes.c_char_p]
            lib.axon_stop_nrt_profile.restype = ctypes.c_int64

            @contextlib.contextmanager
            def hook(output_dir, device_ids):
                import jax

                jax.devices()
                if device_ids:
                    ids = (ctypes.c_int64 * len(device_ids))(*device_ids)
                    rc = lib.axon_start_nrt_profile(ids, len(device_ids))
                else:
                    rc = lib.axon_start_nrt_profile(None, 0)
                if rc != 0:
                    raise RuntimeError(f"axon_start_nrt_profile rc={rc}")
                try:
                    yield
                finally:
                    n = lib.axon_stop_nrt_profile(str(output_dir).encode())
                    if n <= 0:
                        print(
                            f"ntff profile: rc={n} writing {output_dir}",
                            file=sys.stderr,
                        )

    mod = types.ModuleType("antenv.axon_hooks")
    _state = {"hook": hook}
    mod.set_axon_ntff_profile_hook = lambda h: _state.__setitem__("hook", h)
    mod.get_axon_ntff_profile_hook = lambda: _state["hook"]
    sys.modules["antenv.axon_hooks"] = mod
    try:
        import antenv

        antenv.axon_hooks = mod
    except ImportError:
        pass


_ensure_ntff_hook()

import concourse.bass as bass
import concourse.bacc as bacc_mod
import concourse.mybir as mybir
import concourse.tile as tile
from concourse.bass import ts
from concourse.bass_utils import run_bass_kernel_spmd

N_CORES = 8
P = 128
N_TILE = 512  # one fp32 PSUM bank

_nc_cache = {}
LAST_RESULTS = None  # BassKernelResults of the most recent run (for test.py)


def _build_nc(T_pad: int, D: int, O: int):
    KO = D // P
    NO = O // N_TILE
    bf16 = mybir.dt.bfloat16
    f32 = mybir.dt.float32

    # m-tiles: full 128-row tiles plus one remainder tile (multiple of 32)
    m_sizes = [P] * (T_pad // P)
    if T_pad % P:
        m_sizes.append(T_pad % P)
    MO = len(m_sizes)
    m_starts = [sum(m_sizes[:i]) for i in range(MO)]

    nc = bacc_mod.Bacc()
    xT = nc.dram_tensor("xT", [D, T_pad], bf16, kind="ExternalInput")
    w = nc.dram_tensor("w", [D, O], bf16, kind="ExternalInput")
    bias = nc.dram_tensor("bias", [P, O], f32, kind="ExternalInput")
    out = nc.dram_tensor("out", [T_pad, O], bf16, kind="ExternalOutput")

    xT_t = xT[:, :].rearrange("(ko p) t -> p ko t", p=P)
    w_t = w[:, :].rearrange("(ko p) o -> p ko o", p=P)

    # Tile schedule: (m, n) psum groups. Pass A holds 8 groups (all 8
    # PSUM banks) and runs k-outer so the PE consumes each k-slice as
    # its DMA lands; pass B reruns the remaining groups on recycled
    # banks after pass A's drain, resident inputs, staggered finish.
    passA = [(m, 0) for m in range(MO)] + [(m, 1) for m in range(min(3, MO))]
    passA = passA[:8]
    passB = [(m, n) for n in range(NO) for m in range(MO) if (m, n) not in passA]

    with tile.TileContext(nc) as tc:
        with (
            tc.tile_pool(name="resident", bufs=1) as rpool,
            tc.tile_pool(name="psum", bufs=8, space="PSUM") as psum_pool,
            tc.tile_pool(name="obuf", bufs=MO) as opool,
        ):
            ps = {
                mn: psum_pool.tile(
                    [m_sizes[mn[0]], N_TILE], f32, tag="ps", name=f"ps_{mn[0]}_{mn[1]}"
                )
                for mn in passA
            }
            # HAM warm-up: dummy matmuls lift the PE clock gate to 8/8
            # before the real stream starts. Each bass-level warm matmul
            # lowers to 2 MATMUL instructions (measured), so 12 calls =
            # ~2.6 us of PE activity. They target the last pass-A psum
            # group as throwaway singleton groups — the real k=0 matmul
            # (start=True) clears the bank, so no extra bank is burned.
            warm_sb = rpool.tile([P, 64], f32, tag="warm")
            nc.gpsimd.memset(warm_sb[:], 0.0)
            warm_tgt = ps[passA[-1]]
            for i in range(12):
                nc.tensor.matmul(
                    warm_tgt[:64, :64],
                    lhsT=warm_sb[:, :64],
                    rhs=warm_sb[:, :64],
                    start=True,
                    stop=True,
                )
            # Input loads: one DMA per k-slice (x [128, T_pad], w
            # [128, O], both contiguous bf16), alternated across the two
            # HWDGE queues so slice k lands ~k * 1.1 us in — matching the
            # PE's ~1.7 us per k-step burn rate. bias arrives host-tiled
            # as [128, O] and is issued LAST on the scalar queue, so its
            # 512 KB transfers after all x/w slices (it is only needed at
            # the pass-A drain ~6 us later).
            bias_sb = rpool.tile([P, O], f32, tag="bias")
            x_sb = []
            w_sb = []
            for k in range(KO):
                xt = rpool.tile([P, T_pad], bf16, tag=f"x{k}")
                wt = rpool.tile([P, O], bf16, tag=f"w{k}")
                if k % 2 == 0:
                    nc.sync.dma_start(xt[:], xT_t[:, k, :])
                    nc.scalar.dma_start(wt[:], w_t[:, k, :])
                else:
                    nc.scalar.dma_start(xt[:], xT_t[:, k, :])
                    nc.sync.dma_start(wt[:], w_t[:, k, :])
                x_sb.append(xt)
                w_sb.append(wt)
            nc.scalar.dma_start(bias_sb[:], bias[:, :])

            def x_ap(k, m):
                return x_sb[k][:, m_starts[m] : m_starts[m] + m_sizes[m]]

            obufs = [
                opool.tile([P, O], bf16, tag="ot", name=f"ot{m}")
                for m in range(MO)
            ]
            out_written = {m: 0 for m in range(MO)}

            def drain(mn):
                m, n = mn
                nc.vector.tensor_add(
                    obufs[m][: m_sizes[m], ts(n, N_TILE)],
                    ps[mn][:],
                    bias_sb[: m_sizes[m], ts(n, N_TILE)],
                )
                out_written[m] += 1
                if out_written[m] == NO:
                    eng = nc.sync if m % 2 == 0 else nc.scalar
                    eng.dma_start(
                        out[m_starts[m] : m_starts[m] + m_sizes[m], :],
                        obufs[m][: m_sizes[m], :],
                    )

            for k in range(KO):
                for mn in passA:
                    nc.tensor.matmul(
                        ps[mn][:],
                        lhsT=x_ap(k, mn[0]),
                        rhs=w_sb[k][:, ts(mn[1], N_TILE)],
                        start=(k == 0),
                        stop=(k == KO - 1),
                    )
            for mn in passA:
                drain(mn)
            for mn in passB:
                ps[mn] = psum_pool.tile(
                    [m_sizes[mn[0]], N_TILE], f32, tag="ps", name=f"ps_{mn[0]}_{mn[1]}"
                )
                for k in range(KO):
                    nc.tensor.matmul(
                        ps[mn][:],
                        lhsT=x_ap(k, mn[0]),
                        rhs=w_sb[k][:, ts(mn[1], N_TILE)],
                        start=(k == 0),
                        stop=(k == KO - 1),
                    )
                drain(mn)
    nc.finalize()
    return nc


def kernel(x, category_id, weight, bias):
    global LAST_RESULTS
    x = np.asarray(x)
    category_id = np.asarray(category_id)
    weight = np.asarray(weight, dtype=np.float32)
    bias = np.ascontiguousarray(np.asarray(bias), dtype=np.float32)

    orig_shape = x.shape
    D = orig_shape[-1]
    C, _, O = weight.shape
    assert C == N_CORES and D % P == 0 and O % N_TILE == 0

    T = int(np.prod(orig_shape[:-1]))
    x_flat = np.ascontiguousarray(x.reshape(T, D), dtype=np.float32)
    cid = category_id.reshape(T).astype(np.int64)

    idx_per_c = [np.flatnonzero(cid == c) for c in range(C)]
    counts = [len(ix) for ix in idx_per_c]
    T_pad = max(32, -(-max(counts) // 32) * 32)  # multiple of 32 (PE col-group)

    key = (T_pad, D, O)
    if key not in _nc_cache:
        _nc_cache[key] = _build_nc(T_pad, D, O)
    nc = _nc_cache[key]

    w_bf16 = weight.astype(BF16)
    in_maps = []
    for c in range(C):
        xcT = np.zeros((D, T_pad), dtype=BF16)
        xcT[:, : counts[c]] = x_flat[idx_per_c[c]].T.astype(BF16)
        in_maps.append(
            {
                "xT": xcT,
                "w": w_bf16[c],
                "bias": np.ascontiguousarray(
                    np.broadcast_to(bias[c : c + 1], (P, O))
                ),
            }
        )

    res = run_bass_kernel_spmd(nc, in_maps, list(range(N_CORES)))
    LAST_RESULTS = res

    out_flat = np.empty((T, O), dtype=np.float32)
    for c in range(C):
        out_flat[idx_per_c[c]] = res.results[c]["out"][: counts[c]].astype(
            np.float32
        )
    return out_flat.reshape(*orig_shape[:-1], O)
